# revision 1
# baseline (speedup 1.0000x reference)
"""Trainium2 Bass kernel for CornerBoundingBoxEMDLoss.

For each sample: 8x8 pairwise corner distances, then exact min-cost perfect
matching. Instead of brute-forcing all 8! = 40320 permutations (the reference
does a [B,64]@[64,40320] GEMM + row-min), we use meet-in-the-middle:

  min over perms = min over 70 4-subsets T of
      (min assignment of preds {0,1,2,3} onto T)
    + (min assignment of preds {4,5,6,7} onto complement(T))

computed hierarchically: pred pairs -> target pairs (L1, one-hot GEMM with
two orderings + elementwise min), pairs -> quads (L2, one-hot GEMM over
2+2 splits + group-min-of-6), then the complement-aligned A+B pairing with a
fused add+min reduction (L3). Exact same minimum, ~50x less arithmetic.

Data-parallel across 8 NeuronCores: 512 samples per core, processed as
4 chunks of 128 samples (samples on SBUF partitions, transposed to
coord-major via PE transpose for the selection GEMMs).
"""

import itertools

import numpy as np

import concourse.bacc as bacc
import concourse.mybir as mybir
import concourse.tile as tile

N_CORES = 8
B_TOTAL = 4096
B_CORE = B_TOTAL // N_CORES          # 512
N_CHUNKS = 4
CHUNK = B_CORE // N_CHUNKS           # 128

F32 = mybir.dt.float32
# dtype used for the one-hot selection GEMMs (fp32 exact; float32r is 4x
# faster on the PE and exact for 0/1 weights if its decomposition holds --
# verified empirically before enabling).
GEMM_DT = mybir.dt.float32

MIN_INIT = 1.0e30


def _build_constants():
    """One-hot selection matrices for the two GEMM levels."""
    pairs = list(itertools.combinations(range(8), 2))            # 28
    pair_idx = {p: i for i, p in enumerate(pairs)}
    subs4 = list(itertools.combinations(range(8), 4))            # 70
    pred_pairs = [(0, 1), (2, 3), (4, 5), (6, 7)]

    l1o0 = np.zeros((64, 112), dtype=np.float32)
    l1o1 = np.zeros((64, 112), dtype=np.float32)
    for q, (i0, i1) in enumerate(pred_pairs):
        for p, (a, b) in enumerate(pairs):
            col = q * 28 + p
            l1o0[i0 * 8 + a, col] = 1; l1o0[i1 * 8 + b, col] = 1
            l1o1[i0 * 8 + b, col] = 1; l1o1[i1 * 8 + a, col] = 1

    l2 = np.zeros((112, 840), dtype=np.float32)
    for t, T in enumerate(subs4):
        for s, S in enumerate(itertools.combinations(T, 2)):
            rest = tuple(sorted(set(T) - set(S)))
            l2[0 * 28 + pair_idx[S], t * 6 + s] = 1
            l2[1 * 28 + pair_idx[rest], t * 6 + s] = 1
        TB = tuple(sorted(set(range(8)) - set(T)))               # complement
        for s, S in enumerate(itertools.combinations(TB, 2)):
            rest = tuple(sorted(set(TB) - set(S)))
            l2[2 * 28 + pair_idx[S], 420 + t * 6 + s] = 1
            l2[3 * 28 + pair_idx[rest], 420 + t * 6 + s] = 1

    ident = np.eye(128, dtype=np.float32)
    return l1o0, l1o1, l2, ident


def build_nc():
    nc = bacc.Bacc("TRN2", target_bir_lowering=False, debug=False)

    pred_d = nc.dram_tensor("pred", [B_CORE, 24], F32, kind="ExternalInput")
    targn_d = nc.dram_tensor("targn", [B_CORE, 24], F32, kind="ExternalInput")
    l1o0_d = nc.dram_tensor("l1o0", [64, 112], GEMM_DT, kind="ExternalInput")
    l1o1_d = nc.dram_tensor("l1o1", [64, 112], GEMM_DT, kind="ExternalInput")
    l2_d = nc.dram_tensor("l2mat", [112, 840], GEMM_DT, kind="ExternalInput")
    id_d = nc.dram_tensor("ident", [128, 128], F32, kind="ExternalInput")
    out_d = nc.dram_tensor("out", [B_CORE], F32, kind="ExternalOutput")

    with tile.TileContext(nc) as tc:
        with (
            tc.tile_pool(name="consts", bufs=1) as cpool,
            tc.tile_pool(name="persist", bufs=1) as ppool,
            tc.tile_pool(name="work", bufs=3) as wpool,
            tc.tile_pool(name="psum_t", bufs=2, space="PSUM") as pst,
            tc.tile_pool(name="psum_l1", bufs=1, space="PSUM") as psl1,
            tc.tile_pool(name="psum_l2", bufs=2, space="PSUM") as psl2,
        ):
            c_l1o0 = cpool.tile([64, 112], GEMM_DT, tag="l1o0")
            c_l1o1 = cpool.tile([64, 112], GEMM_DT, tag="l1o1")
            c_l2 = cpool.tile([112, 840], GEMM_DT, tag="l2")
            c_id = cpool.tile([128, 128], F32, tag="ident")
            nc.sync.dma_start(c_l1o0[:, :], l1o0_d[:, :])
            nc.sync.dma_start(c_l1o1[:, :], l1o1_d[:, :])
            nc.sync.dma_start(c_l2[:, :], l2_d[:, :])
            nc.sync.dma_start(c_id[:, :], id_d[:, :])

            distT = ppool.tile([64, B_CORE], GEMM_DT, tag="distT")
            m_t = ppool.tile([112, B_CORE], GEMM_DT, tag="m")
            loss = ppool.tile([128, N_CHUNKS], F32, tag="loss")

            # ---- phase 1: pairwise distances, transposed to [64, 512] ----
            for c in range(N_CHUNKS):
                sl = slice(c * CHUNK, (c + 1) * CHUNK)
                p_t = wpool.tile([128, 24], F32, tag="p")
                t_t = wpool.tile([128, 24], F32, tag="t")
                nc.sync.dma_start(p_t[:, :], pred_d[sl, :])
                nc.sync.dma_start(t_t[:, :], targn_d[sl, :])

                # diff[b, i, j, c3] = pred[b, i, c3] + (-target[b, j, c3])
                diff = wpool.tile([128, 192], F32, tag="diff")
                p_b = (p_t[:, :].rearrange("p (i c) -> p i c", i=8)
                       .unsqueeze(2).broadcast_to((128, 8, 8, 3)))
                t_b = (t_t[:, :].rearrange("p (j c) -> p j c", j=8)
                       .unsqueeze(1).broadcast_to((128, 8, 8, 3)))
                d4 = diff[:, :].rearrange("p (i j c) -> p i j c", i=8, j=8)
                nc.gpsimd.tensor_add(d4, p_b, t_b)

                sq = wpool.tile([128, 192], F32, tag="sq")
                nc.scalar.activation(sq[:, :], diff[:, :],
                                     mybir.ActivationFunctionType.Square)

                d2 = wpool.tile([128, 64], F32, tag="d2")
                nc.vector.tensor_reduce(
                    d2[:, :], sq[:, :].rearrange("p (r c) -> p r c", c=3),
                    axis=mybir.AxisListType.X, op=mybir.AluOpType.add)

                tp = pst.tile([64, 128], F32, tag="tp")
                nc.tensor.transpose(tp[:, :], d2[:, :], c_id[:, :])

                # sqrt fused with the PSUM->SBUF copy
                nc.scalar.activation(distT[:, sl], tp[:, :],
                                     mybir.ActivationFunctionType.Sqrt)

            # ---- L1: pred-pair x target-pair costs, both orderings ----
            ps0 = psl1.tile([112, B_CORE], F32, tag="ps0")
            ps1 = psl1.tile([112, B_CORE], F32, tag="ps1")
            nc.tensor.matmul(ps0[:, :], c_l1o0[:, :], distT[:, :],
                             start=True, stop=True)
            nc.tensor.matmul(ps1[:, :], c_l1o1[:, :], distT[:, :],
                             start=True, stop=True)
            # HW: TensorTensor may read at most one input from PSUM
            s1 = ppool.tile([112, B_CORE], F32, tag="s1")
            nc.scalar.activation(s1[:, :], ps1[:, :],
                                 mybir.ActivationFunctionType.Copy)
            nc.vector.tensor_tensor(m_t[:, :], ps0[:, :], s1[:, :],
                                    op=mybir.AluOpType.min)

            # ---- L2 + L3 per chunk ----
            for c in range(N_CHUNKS):
                sl = slice(c * CHUNK, (c + 1) * CHUNK)
                ps2 = psl2.tile([128, 1024], F32, tag="ps2")
                nc.tensor.matmul(ps2[:, 0:420], m_t[:, sl], c_l2[:, 0:420],
                                 start=True, stop=True)
                nc.tensor.matmul(ps2[:, 512:932], m_t[:, sl], c_l2[:, 420:840],
                                 start=True, stop=True)

                minab = wpool.tile([128, 140], F32, tag="minab")
                v = (ps2[:, :].rearrange("p (h x) -> p h x", h=2)[:, :, 0:420]
                     .rearrange("p h (t s) -> p h t s", s=6))
                nc.vector.tensor_reduce(minab[:, :], v,
                                        axis=mybir.AxisListType.X,
                                        op=mybir.AluOpType.min)

                scratch = wpool.tile([128, 70], F32, tag="scratch")
                nc.vector.tensor_tensor(scratch[:, :], minab[:, 0:70],
                                        minab[:, 70:140],
                                        op=mybir.AluOpType.add)
                nc.vector.tensor_reduce(loss[:, c:c + 1], scratch[:, :],
                                        axis=mybir.AxisListType.X,
                                        op=mybir.AluOpType.min)

            # loss[p, c] -> out[c*128 + p]
            nc.sync.dma_start(
                out_d[:].rearrange("(c p) -> p c", p=128), loss[:, :])

    nc.compile()
    return nc


_NC = None


def _get_nc():
    global _NC
    if _NC is None:
        _NC = build_nc()
    return _NC


def kernel(pred_corners: np.ndarray, target_corners: np.ndarray) -> np.ndarray:
    from concourse.bass_utils import run_bass_kernel_spmd

    nc = _get_nc()
    l1o0, l1o1, l2, ident = _build_constants()
    pred = np.ascontiguousarray(pred_corners, dtype=np.float32).reshape(B_TOTAL, 24)
    targn = -np.ascontiguousarray(target_corners, dtype=np.float32).reshape(B_TOTAL, 24)

    in_maps = []
    for k in range(N_CORES):
        sl = slice(k * B_CORE, (k + 1) * B_CORE)
        in_maps.append({
            "pred": pred[sl], "targn": targn[sl],
            "l1o0": l1o0, "l1o1": l1o1, "l2mat": l2, "ident": ident,
        })

    res = run_bass_kernel_spmd(nc, in_maps, core_ids=list(range(N_CORES)))
    return np.concatenate([res.results[k]["out"] for k in range(N_CORES)])



# revision 9
# speedup vs baseline: 1.8749x; 1.8749x over previous
"""Trainium2 Bass kernel for CornerBoundingBoxEMDLoss.

For each sample: 8x8 pairwise corner distances, then exact min-cost perfect
matching via meet-in-the-middle:

  min over perms = min over 70 4-subsets T of
      (min assignment of preds {0,1,2,3} onto T)
    + (min assignment of preds {4,5,6,7} onto complement(T))

computed hierarchically: pred pairs -> target pairs (L1, one-hot GEMM with
two orderings + elementwise min), pairs -> quads (L2, one-hot GEMM over the
6 = C(4,2) pair-to-half assignments per 2+2 split + group-min), then a fused
add+min reduction over the 70 complement-aligned A+B sums (L3). Exact same
minimum as brute force over 8! permutations, ~50x less arithmetic.

Data-parallel across 8 NeuronCores: 512 samples per core, processed as
4 chunks of 128 samples (samples on SBUF partitions; d2 rows of two chunks
are transposed together in one PE pass to coord-major for the selection
GEMMs). Selection GEMMs run in bf16 (one-hot weights are exact in bf16;
distances round to ~0.4% which is well inside the 2e-2 gate). All inputs
arrive in 3 packed DMAs; the output leaves as one [128,4] DMA that the host
reorders.
"""

import itertools

import numpy as np
import ml_dtypes

import concourse.bacc as bacc
import concourse.mybir as mybir
import concourse.tile as tile

N_CORES = 8
B_TOTAL = 4096
B_CORE = B_TOTAL // N_CORES          # 512
N_CHUNKS = 4
CHUNK = B_CORE // N_CHUNKS           # 128

F32 = mybir.dt.float32
BF16 = mybir.dt.bfloat16

MIN_INIT = 1.0e30


def _build_constants():
    """Packed one-hot selection matrices + identity.

    cpack [128, 1064] bf16:
      cols   0:112  l1 ordering 0   (partitions 0:64 and replicated 64:128)
      cols 112:224  l1 ordering 1   (same replication)
      cols 224:1064 l2 (partitions 0:112): 840 = [A-side 70*6 | B-side 70*6]
    ident [128, 128] f32 for PE transposes.
    """
    pairs = list(itertools.combinations(range(8), 2))            # 28
    pair_idx = {p: i for i, p in enumerate(pairs)}
    subs4 = list(itertools.combinations(range(8), 4))            # 70
    pred_pairs = [(0, 1), (2, 3), (4, 5), (6, 7)]

    l1o0 = np.zeros((64, 112), dtype=np.float32)
    l1o1 = np.zeros((64, 112), dtype=np.float32)
    for q, (i0, i1) in enumerate(pred_pairs):
        for p, (a, b) in enumerate(pairs):
            col = q * 28 + p
            l1o0[i0 * 8 + a, col] = 1; l1o0[i1 * 8 + b, col] = 1
            l1o1[i0 * 8 + b, col] = 1; l1o1[i1 * 8 + a, col] = 1

    # all 6 C(T,2) choices of which target pair the first pred pair gets
    # (each 2+2 split appears twice with the pair roles swapped -- those are
    # distinct matchings, both needed)
    l2 = np.zeros((112, 840), dtype=np.float32)
    for t, T in enumerate(subs4):
        for s, S in enumerate(itertools.combinations(T, 2)):
            R = tuple(sorted(set(T) - set(S)))
            l2[0 * 28 + pair_idx[S], t * 6 + s] = 1
            l2[1 * 28 + pair_idx[R], t * 6 + s] = 1
        TB = tuple(sorted(set(range(8)) - set(T)))               # complement
        for s, S in enumerate(itertools.combinations(TB, 2)):
            R = tuple(sorted(set(TB) - set(S)))
            l2[2 * 28 + pair_idx[S], 420 + t * 6 + s] = 1
            l2[3 * 28 + pair_idx[R], 420 + t * 6 + s] = 1

    cpack = np.zeros((128, 1064), dtype=np.float32)
    cpack[0:64, 0:112] = l1o0
    cpack[0:64, 112:224] = l1o1
    cpack[64:128, 0:224] = cpack[0:64, 0:224]
    cpack[0:112, 224:1064] = l2
    cpack = cpack.astype(ml_dtypes.bfloat16)

    ident = np.eye(128, dtype=np.float32)
    return cpack, ident


def build_nc():
    nc = bacc.Bacc("TRN2", target_bir_lowering=False, debug=False)

    # data: per chunk c the 48-col block [pred_c | -targ_c], see kernel()
    data_d = nc.dram_tensor("data", [CHUNK, 48 * N_CHUNKS], F32,
                            kind="ExternalInput")
    cpack_d = nc.dram_tensor("cpack", [128, 1064], BF16, kind="ExternalInput")
    id_d = nc.dram_tensor("ident", [128, 128], F32, kind="ExternalInput")
    out_d = nc.dram_tensor("out", [CHUNK, N_CHUNKS], F32, kind="ExternalOutput")

    with tile.TileContext(nc) as tc:
        with (
            tc.tile_pool(name="consts", bufs=1) as cpool,
            tc.tile_pool(name="persist", bufs=1) as ppool,
            tc.tile_pool(name="work", bufs=3) as wpool,
            tc.tile_pool(name="pairs", bufs=2) as qpool,
            tc.tile_pool(name="psum_t", bufs=1, space="PSUM") as pst,
            tc.tile_pool(name="psum_a", bufs=2, space="PSUM") as psa,
            tc.tile_pool(name="psum_l2", bufs=2, space="PSUM") as psl2,
        ):
            data = cpool.tile([CHUNK, 192], F32, tag="data")
            cpk = cpool.tile([128, 1064], BF16, tag="cpack")
            c_id = cpool.tile([128, 128], F32, tag="ident")
            # data first (gates the whole pipeline), on the sync queue;
            # consts on the scalar queue in parallel; ident second on sync.
            nc.sync.dma_start(data[:, :], data_d[:, :])
            nc.scalar.dma_start(cpk[:, :], cpack_d[:, :])
            nc.sync.dma_start(c_id[:, :], id_d[:, :])

            m_t = ppool.tile([112, B_CORE], BF16, tag="m")
            loss = ppool.tile([128, N_CHUNKS], F32, tag="loss")
            tiny = ppool.tile([1, 1], F32, tag="tiny")

            # Force the (single) act table that holds sqrt+square+copy: the
            # table-load pass picks the table of the first activation, and
            # sqrt_and_others covers everything we use.
            nc.gpsimd.memset(tiny[:, :], 1.0)
            nc.scalar.activation(tiny[:, :], tiny[:, :],
                                 mybir.ActivationFunctionType.Sqrt)

            d2p = [None, None]
            dtp = [None, None]

            def phase1(c):
                """distances^2 for chunk c -> d2 pair tile column half."""
                pair, half = divmod(c, 2)
                if half == 0:
                    d2p[pair] = qpool.tile([CHUNK, 128], F32, tag="d2p", name="d2p")
                dsl = data[:, 48 * c: 48 * c + 48]
                diff = wpool.tile([CHUNK, 192], F32, tag="diff")
                p_b = (dsl[:, 0:24].rearrange("p (i c) -> p i c", i=8)
                       .unsqueeze(2).broadcast_to((CHUNK, 8, 8, 3)))
                t_b = (dsl[:, 24:48].rearrange("p (j c) -> p j c", j=8)
                       .unsqueeze(1).broadcast_to((CHUNK, 8, 8, 3)))
                d4 = diff[:, :].rearrange("p (i j c) -> p i j c", i=8, j=8)
                nc.gpsimd.tensor_add(d4, p_b, t_b)

                sq = wpool.tile([CHUNK, 192], BF16, tag="sq")
                nc.scalar.activation(sq[:, :], diff[:, :],
                                     mybir.ActivationFunctionType.Square)
                nc.vector.tensor_reduce(
                    d2p[pair][:, 64 * half: 64 * half + 64],
                    sq[:, :].rearrange("p (r c) -> p r c", c=3),
                    axis=mybir.AxisListType.X, op=mybir.AluOpType.add)

            def transpose_pair(pair):
                """[128 samples, 2x64 d2] -> bf16 dist [2x64, 128 samples]."""
                tp = pst.tile([128, 128], F32, tag="tp")
                nc.tensor.transpose(tp[:, :], d2p[pair][:, :], c_id[:, :])
                dtp[pair] = qpool.tile([128, 128], BF16, tag="dtp", name="dtp")
                nc.scalar.activation(dtp[pair][:, :], tp[:, :],
                                     mybir.ActivationFunctionType.Sqrt)

            def l1(c):
                """pred-pair x target-pair costs for chunk c -> m_t cols."""
                pair, half = divmod(c, 2)
                hp = slice(64 * half, 64 * half + 64)
                rhs = dtp[pair][hp, :]
                ps01 = psa.tile([112, 256], F32, tag="ps01")
                nc.tensor.matmul(ps01[:, 0:128], cpk[hp, 0:112], rhs,
                                 start=True, stop=True)
                nc.tensor.matmul(ps01[:, 128:256], cpk[hp, 112:224], rhs,
                                 start=True, stop=True)
                # HW: TensorTensor may read at most one input from PSUM
                s1 = wpool.tile([112, 128], F32, tag="s1")
                nc.scalar.activation(s1[:, :], ps01[:, 128:256],
                                     mybir.ActivationFunctionType.Copy)
                nc.vector.tensor_tensor(
                    m_t[:, CHUNK * c: CHUNK * (c + 1)], ps01[:, 0:128],
                    s1[:, :], op=mybir.AluOpType.min)

            def l2l3(c):
                """quad costs + final min for chunk c."""
                ps2 = psl2.tile([128, 1024], F32, tag="ps2")
                msl = m_t[:, CHUNK * c: CHUNK * (c + 1)]
                nc.tensor.matmul(ps2[:, 0:420], msl, cpk[0:112, 224:644],
                                 start=True, stop=True)
                nc.tensor.matmul(ps2[:, 512:932], msl, cpk[0:112, 644:1064],
                                 start=True, stop=True)
                minab = wpool.tile([128, 140], BF16, tag="minab")
                v = (ps2[:, :].rearrange("p (h x) -> p h x", h=2)[:, :, 0:420]
                     .rearrange("p h (t s) -> p h t s", s=6))
                nc.vector.tensor_reduce(
                    minab[:, :], v,
                    axis=mybir.AxisListType.X, op=mybir.AluOpType.min)
                scratch = wpool.tile([128, 70], BF16, tag="scratch")
                nc.vector.tensor_tensor(scratch[:, :], minab[:, 0:70],
                                        minab[:, 70:140],
                                        op=mybir.AluOpType.add)
                nc.vector.tensor_reduce(loss[:, c:c + 1], scratch[:, :],
                                        axis=mybir.AxisListType.X,
                                        op=mybir.AluOpType.min)

            # pipelined schedule (engine streams stay dependency-ordered):
            phase1(0); phase1(1)
            transpose_pair(0)
            phase1(2); phase1(3)
            l1(0); l1(1)
            transpose_pair(1)
            l2l3(0); l2l3(1)
            l1(2); l1(3)
            l2l3(2); l2l3(3)

            # loss[p, c] -> dram [p, c]; host reorders to c*128+p
            nc.sync.dma_start(out_d[:, :], loss[:, :])

    nc.compile()
    return nc


_NC = None


def _get_nc():
    global _NC
    if _NC is None:
        _NC = build_nc()
    return _NC


def _input_maps(pred_corners, target_corners):
    cpack, ident = _build_constants()
    pred = np.ascontiguousarray(pred_corners, dtype=np.float32)
    targ = np.ascontiguousarray(target_corners, dtype=np.float32)
    in_maps = []
    for k in range(N_CORES):
        sl = slice(k * B_CORE, (k + 1) * B_CORE)
        # [4 chunks, 128 slots, 24] -> [128, 4*48] with per-chunk blocks
        # [pred_c | -targ_c]
        pk = pred[sl].reshape(N_CHUNKS, CHUNK, 24)
        tk = targ[sl].reshape(N_CHUNKS, CHUNK, 24)
        datak = np.concatenate([pk, -tk], axis=2)          # [4, 128, 48]
        datak = np.ascontiguousarray(
            datak.transpose(1, 0, 2).reshape(CHUNK, 192))
        in_maps.append({"data": datak, "cpack": cpack, "ident": ident})
    return in_maps


def _gather(results):
    outs = []
    for k in range(N_CORES):
        o = results[k]["out"].reshape(CHUNK, N_CHUNKS)
        outs.append(np.ascontiguousarray(o.T).reshape(B_CORE))
    return np.concatenate(outs)


def kernel(pred_corners: np.ndarray, target_corners: np.ndarray) -> np.ndarray:
    from concourse.bass_utils import run_bass_kernel_spmd

    nc = _get_nc()
    in_maps = _input_maps(pred_corners, target_corners)
    res = run_bass_kernel_spmd(nc, in_maps, core_ids=list(range(N_CORES)))
    return _gather(res.results)
